# revision 24
# baseline (speedup 1.0000x reference)
"""Trainium2 Bass kernel for a seq2seq decoder step (Bahdanau attention +
2-layer LSTM cell + vocab projection), SPMD across 8 NeuronCores.

Sharding (per core c):
  - attention: batch-parallel (8 of 64 batch rows per core)
  - LSTM gates: H-parallel (128 of 1024 hidden units per core, all batches)
  - output projection: vocab-parallel (4000 of 32000 rows per core)
  - AllGather collectives stitch context (batch axis) and h0/h1 (H axis).

All activations are kept feature-major ("T layout", [feature, batch]) so
every matmul contraction lands on the partition axis; all weights are
pre-transposed on the host while sharding (contraction dim first).
"""

import sys
from contextlib import ExitStack

if "/opt/trn_rl_repo" not in sys.path:
    sys.path.insert(0, "/opt/trn_rl_repo")

import numpy as np

V, E, H, ENC, L, B, S = 32000, 512, 1024, 1024, 2, 64, 256
NCORES = 8
BL = B // NCORES        # 8  local batches (attention shard)
HL = H // NCORES        # 128 local hidden slice (LSTM shard)
VL = V // NCORES        # 4000 local vocab slice (output shard)
NEG = -1.0e10


# ---------------------------------------------------------------------------
# Tail-drain workaround: this walrus build rejects >1 sync-wait command on
# CTRL-less instructions (Drain/NoOp). Spread the kernel-tail waits over
# dedicated sync-engine NOPs, one wait each.
# ---------------------------------------------------------------------------
def _patched_tile_context(tile_mod, mybir):
    from concourse.vector_clock import ScopedClock

    class PatchedTileContext(tile_mod.TileContext):
        def _drain_and_barrier(self, tick_clock, wait_clock):
            nc = self.nc
            probe = nc.sync.nop(nofuse=True)
            wait_clock.add_sem_waits(
                probe.ins, ScopedClock({None: tick_clock.global_clock})
            )
            si = probe.ins.sync_info
            waits = list(si.on_wait) if si is not None else []
            updates = list(si.on_update) if si is not None else []
            probe.ins.sync_info = mybir.SyncInfo(
                on_wait=waits[:1], on_update=updates
            )
            for i in range(1, len(waits)):
                n = nc.sync.nop(nofuse=True)
                n.ins.sync_info = mybir.SyncInfo(
                    on_wait=waits[i : i + 1], on_update=[]
                )
            nc.sync.drain()
            nc.all_engine_barrier()
            assert self.sems is not None
            popped = nc._tile_sem_poison_stack.pop()
            assert popped is self._sem_poison
            nc.clear_and_free_semaphores(list(self.sems.allocated().values()))
            nc.all_engine_barrier()

    return PatchedTileContext


# ---------------------------------------------------------------------------
# Program construction (identical on every core; all per-core variation is
# carried by the input data).
# ---------------------------------------------------------------------------
def _split_multi_waits(nc, mybir):
    """This walrus build accepts at most one sync-wait command per
    instruction; Tile attaches several.  Move extra waits onto dedicated
    same-engine NOPs placed immediately before the instruction."""
    for bb in nc.main_func.blocks:
        out = []
        changed = False
        for ins in bb.instructions:
            si = ins.sync_info
            waits = list(si.on_wait) if si is not None else []
            if len(waits) > 1:
                changed = True
                for j, w in enumerate(waits[:-1]):
                    nop = mybir.InstNoOp(
                        name=f"{ins.name}_w{j}",
                        engine=ins.engine,
                        sync_info=mybir.SyncInfo(on_wait=[w], on_update=[]),
                        bass_nofuse=True,
                    )
                    nc.register_instruction(nop, overwrite=True)
                    out.append(nop)
                ins.sync_info = mybir.SyncInfo(
                    on_wait=[waits[-1]], on_update=list(si.on_update)
                )
            out.append(ins)
        if changed:
            bb.instructions = out


def _build_program():
    import concourse.bass as bass
    import concourse.mybir as mybir
    import concourse.tile as tile

    f32 = mybir.dt.float32
    Alu = mybir.AluOpType
    Act = mybir.ActivationFunctionType
    TC = _patched_tile_context(tile, mybir)

    nc = bass.Bass()

    # ---- per-core inputs -------------------------------------------------
    encT = nc.dram_tensor("encT", [ENC, BL, S], f32, kind="ExternalInput")
    maskbias = nc.dram_tensor("maskbias", [BL * S], f32, kind="ExternalInput")
    embT = nc.dram_tensor("embT", [E, B], f32, kind="ExternalInput")
    h0inT = nc.dram_tensor("h0inT", [H, B], f32, kind="ExternalInput")
    h1inT = nc.dram_tensor("h1inT", [H, B], f32, kind="ExternalInput")
    hq1T = nc.dram_tensor("hq1T", [H, BL], f32, kind="ExternalInput")
    c0inT = nc.dram_tensor("c0inT", [HL, B], f32, kind="ExternalInput")
    c1inT = nc.dram_tensor("c1inT", [HL, B], f32, kind="ExternalInput")
    wqT = nc.dram_tensor("wqT", [H, H], f32, kind="ExternalInput")
    wkT = nc.dram_tensor("wkT", [ENC, H], f32, kind="ExternalInput")
    vvec = nc.dram_tensor("vvec", [H], f32, kind="ExternalInput")
    wih0T = nc.dram_tensor("wih0T", [E + ENC, 4 * HL], f32, kind="ExternalInput")
    whh0T = nc.dram_tensor("whh0T", [H, 4 * HL], f32, kind="ExternalInput")
    wih1T = nc.dram_tensor("wih1T", [H, 4 * HL], f32, kind="ExternalInput")
    whh1T = nc.dram_tensor("whh1T", [H, 4 * HL], f32, kind="ExternalInput")
    bias0 = nc.dram_tensor("bias0", [HL, 4], f32, kind="ExternalInput")
    bias1 = nc.dram_tensor("bias1", [HL, 4], f32, kind="ExternalInput")
    woutT = nc.dram_tensor("woutT", [ENC + E + H, VL], f32, kind="ExternalInput")
    boutv = nc.dram_tensor("boutv", [VL], f32, kind="ExternalInput")

    # ---- per-core outputs ------------------------------------------------
    logits_o = nc.dram_tensor("logits", [B, VL], f32, kind="ExternalOutput")
    attnw_o = nc.dram_tensor("attnw", [BL, S], f32, kind="ExternalOutput")
    h0_o = nc.dram_tensor("h0o", [HL, B], f32, kind="ExternalOutput")
    h1_o = nc.dram_tensor("h1o", [HL, B], f32, kind="ExternalOutput")
    c0_o = nc.dram_tensor("c0o", [HL, B], f32, kind="ExternalOutput")
    c1_o = nc.dram_tensor("c1o", [HL, B], f32, kind="ExternalOutput")

    KT_Q = H // 128        # 8  contraction tiles for Wq/Wk (e dim)
    MT_Q = H // 128        # 8  output tiles for query/keys h dim
    ET = ENC // 128        # 8
    KT_X0 = (E + ENC) // 128   # 12
    KT_H = H // 128        # 8
    KT_F = (ENC + E + H) // 128  # 20 feat tiles, order (ctx, emb, h1)
    NVC = 8                # vocab column chunks of <=512
    GRP = 2                # attention batch groups
    GB = BL // GRP         # 4 batches per group

    with TC(nc) as tc, ExitStack() as root:
        pc = root.enter_context(tc.tile_pool(name="const", bufs=1))
        pdram = root.enter_context(tc.tile_pool(name="dram", bufs=1, space="DRAM"))
        # scopes close in LIFO order: attn first (after phase 3), then lstm
        lstm_scope = ExitStack()
        plstm = lstm_scope.enter_context(tc.tile_pool(name="lstm", bufs=2))
        pwl = lstm_scope.enter_context(tc.tile_pool(name="wl", bufs=4))
        # one PSUM pool for attention + LSTM; every accumulation stream gets
        # its own bank (start=True clears has_written bank-wide on this HW)
        pp_main = lstm_scope.enter_context(
            tc.tile_pool(name="psmain", bufs=2, space="PSUM")
        )
        attn_scope = ExitStack()
        pattn = attn_scope.enter_context(tc.tile_pool(name="attn", bufs=2))
        penc = attn_scope.enter_context(tc.tile_pool(name="enc", bufs=3))

        # ---- phase 0: constants / small loads (sync ring first) ----------
        ones_sb = pc.tile([1, 128], f32, tag="ones")
        nc.vector.memset(ones_sb[:], 1.0)

        mb_sb = pc.tile([1, BL * S], f32, tag="mb")
        nc.sync.dma_start(mb_sb[:], maskbias[:].unsqueeze(0))
        v_sb = pc.tile([128, KT_Q], f32, tag="v")
        nc.sync.dma_start(v_sb[:], vvec[:].rearrange("(kt p) -> p kt", p=128))
        embT_sb = pc.tile([128, E // 128, B], f32, tag="embT")
        nc.sync.dma_start(
            embT_sb[:], embT[:].rearrange("(kt p) b -> p kt b", p=128)
        )
        h0inT_sb = pc.tile([128, KT_H, B], f32, tag="h0inT")
        nc.sync.dma_start(
            h0inT_sb[:], h0inT[:].rearrange("(kt p) b -> p kt b", p=128)
        )
        h1inT_sb = pc.tile([128, KT_H, B], f32, tag="h1inT")
        nc.sync.dma_start(
            h1inT_sb[:], h1inT[:].rearrange("(kt p) b -> p kt b", p=128)
        )
        hq1T_sb = pc.tile([128, KT_H, BL], f32, tag="hq1T")
        nc.sync.dma_start(
            hq1T_sb[:], hq1T[:].rearrange("(kt p) b -> p kt b", p=128)
        )
        c0inT_sb = pc.tile([HL, B], f32, tag="c0inT")
        nc.sync.dma_start(c0inT_sb[:], c0inT[:])
        c1inT_sb = pc.tile([HL, B], f32, tag="c1inT")
        nc.sync.dma_start(c1inT_sb[:], c1inT[:])
        bias0_sb = pc.tile([HL, 4], f32, tag="bias0")
        nc.sync.dma_start(bias0_sb[:], bias0[:])
        bias1_sb = pc.tile([HL, 4], f32, tag="bias1")
        nc.sync.dma_start(bias1_sb[:], bias1[:])

        # ---- big streaming loads (sync ring, in stream order) ------------
        wk_sb = pc.tile([128, KT_Q, H], f32, tag="wk")
        nc.sync.dma_start(wk_sb[:], wkT[:].rearrange("(kt p) m -> p kt m", p=128))
        enc_r = encT[:].rearrange("(et p) b s -> p et b s", p=128)
        enc_halves = []
        for gh in range(BL // 2):  # 4 half-tiles of 2 batches
            eh = penc.tile([128, ET, 2, S], f32, tag="encg", name=f"ench{gh}")
            nc.sync.dma_start(eh[:], enc_r[:, :, 2 * gh : 2 * gh + 2, :])
            enc_halves.append(eh)

        # ---- phase 1: queryT = Wq @ h1inT  -> [H, B] ---------------------
        wq_sb = pattn.tile([128, KT_Q, H], f32, tag="wq", bufs=1)
        nc.sync.dma_start(wq_sb[:], wqT[:].rearrange("(kt p) m -> p kt m", p=128))
        qT_sb = pc.tile([128, MT_Q, BL], f32, tag="qT")
        for mt in range(MT_Q):
            pq = pp_main.tile([128, BL], f32, tag="pk", bufs=5, name=f"pq{mt}")
            for kt in range(KT_Q):
                nc.tensor.matmul(
                    pq[:],
                    wq_sb[:, kt, mt * 128 : (mt + 1) * 128],
                    hq1T_sb[:, kt, :],
                    start=(kt == 0),
                    stop=(kt == KT_Q - 1),
                )
            nc.scalar.activation(qT_sb[:, mt, :], pq[:], Act.Copy)

        # ---- phase 2: attention, 2 groups of 4 batches -------------------
        ctxT_sb = pc.tile([128, ET, BL], f32, tag="ctxT")
        for g in range(GRP):
            pe = [
                pp_main.tile([1, 512], f32, tag="pe", bufs=2, name=f"pe{g}_{i}")
                for i in range(2)
            ]
            # mask bias seeds the energy accumulation
            for p2 in range(2):
                off = (g * GB + 2 * p2) * S
                nc.tensor.matmul(
                    pe[p2][:],
                    ones_sb[0:1, 0:1],
                    mb_sb[0:1, off : off + 2 * S],
                    start=True,
                    stop=False,
                )
            for ht in range(MT_Q):
                pk4 = [
                    pp_main.tile([128, S], f32, tag="pk", bufs=5,
                                 name=f"pk{g}_{ht}_{b4}")
                    for b4 in range(GB)
                ]
                for et in range(ET):
                    for b4 in range(GB):
                        eh = enc_halves[g * 2 + b4 // 2]
                        nc.tensor.matmul(
                            pk4[b4][:],
                            wk_sb[:, et, ht * 128 : (ht + 1) * 128],
                            eh[:, et, b4 % 2, :],
                            start=(et == 0),
                            stop=(et == ET - 1),
                        )
                tanh_t = pattn.tile([128, GB, S], f32, tag="tanh",
                                    name=f"tanh{g}_{ht}")
                for b4 in range(GB):
                    bloc = g * GB + b4
                    nc.scalar.activation(
                        tanh_t[:, b4, :],
                        pk4[b4][:],
                        Act.Tanh,
                        bias=qT_sb[:, ht, bloc : bloc + 1],
                    )
                for p2 in range(2):
                    nc.tensor.matmul(
                        pe[p2][:],
                        v_sb[:, ht : ht + 1],
                        tanh_t[:, 2 * p2 : 2 * p2 + 2, :],
                        start=False,
                        stop=(ht == MT_Q - 1),
                    )
            # softmax over s (masked terms exp to exactly 0)
            mexp = pattn.tile([1, GB, S], f32, tag="mexp", name=f"mexp{g}")
            for b4 in range(GB):
                nc.scalar.activation(
                    mexp[0:1, b4, :],
                    pe[b4 // 2][0:1, (b4 % 2) * S : (b4 % 2 + 1) * S],
                    Act.Exp,
                )
            sums = pattn.tile([1, GB], f32, tag="sums", name=f"sums{g}")
            nc.vector.tensor_reduce(
                sums[:], mexp[:], mybir.AxisListType.X, Alu.add
            )
            recip = pattn.tile([1, GB], f32, tag="recip", name=f"recip{g}")
            nc.vector.reciprocal(recip[:], sums[:])
            atw = pattn.tile([1, GB, S], f32, tag="atw", name=f"atw{g}")
            for b4 in range(GB):
                nc.vector.tensor_scalar_mul(
                    atw[0:1, b4, :],
                    mexp[0:1, b4, :],
                    recip[0:1, b4 : b4 + 1],
                )
            nc.scalar.dma_start(
                attnw_o[g * GB : (g + 1) * GB, :].unsqueeze(0), atw[:]
            )
            # context: ctxT[:, b] = sum_s encT[:, s] * attnw[s]
            for b4 in range(GB):
                bloc = g * GB + b4
                pbc = pp_main.tile([128, S], f32, tag="pbc", bufs=1,
                                   name=f"pbc{bloc}")
                nc.tensor.matmul(
                    pbc[:], ones_sb[0:1, :], atw[0:1, b4, :],
                    start=True, stop=True,
                )
                atw_bc = pattn.tile([128, S], f32, tag="atwbc",
                                    name=f"atwbc{bloc}")
                nc.scalar.activation(atw_bc[:], pbc[:], Act.Copy)
                eh = enc_halves[g * 2 + b4 // 2]
                for et in range(ET):
                    scr = pattn.tile([128, S], f32, tag="scr",
                                     name=f"scr{bloc}_{et}")
                    nc.vector.tensor_tensor(
                        scr[:], eh[:, et, b4 % 2, :], atw_bc[:], Alu.mult
                    )
                    nc.vector.tensor_reduce(
                        ctxT_sb[:, et, bloc : bloc + 1],
                        scr[:],
                        mybir.AxisListType.X,
                        Alu.add,
                    )

        # ---- phase 3: AllGather context over batch -----------------------
        ctx_in = pdram.tile([ENC, BL], f32, tag="ctx_in")
        ctx_all = pdram.tile([NCORES * ENC, BL], f32, tag="ctx_all")
        nc.scalar.dma_start(
            ctx_in[:].rearrange("(et p) b -> p et b", p=128), ctxT_sb[:]
        )
        nc.gpsimd.collective_compute(
            "AllGather",
            Alu.bypass,
            replica_groups=[list(range(NCORES))],
            ins=[ctx_in.opt()],
            outs=[ctx_all.opt()],
        )
        xctx_sb = pc.tile([128, ET, NCORES, BL], f32, tag="xctx")
        ctx_all_r = ctx_all[:].rearrange("(r et p) b -> p et r b", p=128, et=ET)
        for et in range(ET):
            nc.scalar.dma_start(xctx_sb[:, et, :, :], ctx_all_r[:, et, :, :])
        # attention scratch no longer needed; free its SBUF/PSUM for phase 6
        attn_scope.close()

        # ---- phase 4/5: the two LSTM layers (H-sharded gates) ------------
        def lstm_layer(lname, wihT_d, kt_ih, x_tiles, whhT_d, hin_sb, cin_sb,
                       bias_sb, h_out_dram, c_out_dram, h_bounce):
            psg = [
                pp_main.tile([128, B], f32, tag="pk", bufs=5,
                             name=f"psg{lname}_{gs}")
                for gs in range(4)
            ]
            wih_r = wihT_d[:].rearrange("(kt p) m -> p kt m", p=128)
            whh_r = whhT_d[:].rearrange("(kt p) m -> p kt m", p=128)
            for kt in range(kt_ih + KT_H):
                wl = pwl.tile([128, 4 * HL], f32, tag="wl",
                              name=f"wl{lname}_{kt}")
                if kt < kt_ih:
                    nc.sync.dma_start(wl[:], wih_r[:, kt, :])
                else:
                    nc.sync.dma_start(wl[:], whh_r[:, kt - kt_ih, :])
                rhs = x_tiles(kt) if kt < kt_ih else hin_sb[:, kt - kt_ih, :]
                for gs in range(4):
                    nc.tensor.matmul(
                        psg[gs][:],
                        wl[:, gs * HL : (gs + 1) * HL],
                        rhs,
                        start=(kt == 0),
                        stop=(kt == kt_ih + KT_H - 1),
                    )
            gi = plstm.tile([128, B], f32, tag="gi", name=f"gi{lname}")
            gf = plstm.tile([128, B], f32, tag="gf", name=f"gf{lname}")
            gg = plstm.tile([128, B], f32, tag="gg", name=f"gg{lname}")
            go = plstm.tile([128, B], f32, tag="go", name=f"go{lname}")
            nc.scalar.activation(gi[:], psg[0][:], Act.Sigmoid,
                                 bias=bias_sb[:, 0:1])
            nc.scalar.activation(gf[:], psg[1][:], Act.Sigmoid,
                                 bias=bias_sb[:, 1:2])
            nc.scalar.activation(gg[:], psg[2][:], Act.Tanh,
                                 bias=bias_sb[:, 2:3])
            nc.scalar.activation(go[:], psg[3][:], Act.Sigmoid,
                                 bias=bias_sb[:, 3:4])
            fc = plstm.tile([128, B], f32, tag="fc", name=f"fc{lname}")
            nc.vector.tensor_tensor(fc[:], gf[:], cin_sb[:], Alu.mult)
            ig = plstm.tile([128, B], f32, tag="ig", name=f"ig{lname}")
            nc.vector.tensor_tensor(ig[:], gi[:], gg[:], Alu.mult)
            cT = plstm.tile([128, B], f32, tag="cT", name=f"cT{lname}")
            nc.vector.tensor_tensor(cT[:], fc[:], ig[:], Alu.add)
            tc_ = plstm.tile([128, B], f32, tag="tc_", name=f"tc{lname}")
            nc.scalar.activation(tc_[:], cT[:], Act.Tanh)
            hT = plstm.tile([128, B], f32, tag="hT", name=f"hT{lname}")
            nc.vector.tensor_tensor(hT[:], go[:], tc_[:], Alu.mult)
            nc.scalar.dma_start(c_out_dram[:], cT[:])
            nc.scalar.dma_start(h_out_dram[:], hT[:])
            nc.scalar.dma_start(h_bounce[:], hT[:])

        h0_in = pdram.tile([HL, B], f32, tag="h0_in")
        h0_all = pdram.tile([H, B], f32, tag="h0_all")
        lstm_layer(
            "0", wih0T, KT_X0,
            lambda kt: embT_sb[:, kt, :] if kt < 4
            else xctx_sb[:, kt - 4, :, :],
            whh0T, h0inT_sb, c0inT_sb, bias0_sb, h0_o, c0_o, h0_in,
        )
        nc.gpsimd.collective_compute(
            "AllGather",
            Alu.bypass,
            replica_groups=[list(range(NCORES))],
            ins=[h0_in.opt()],
            outs=[h0_all.opt()],
        )
        h0all_sb = pc.tile([128, KT_H, B], f32, tag="h0all")
        nc.scalar.dma_start(
            h0all_sb[:], h0_all[:].rearrange("(kt p) b -> p kt b", p=128)
        )

        h1_in = pdram.tile([HL, B], f32, tag="h1_in")
        h1_all = pdram.tile([H, B], f32, tag="h1_all")
        lstm_layer(
            "1", wih1T, KT_H,
            lambda kt: h0all_sb[:, kt, :],
            whh1T, h1inT_sb, c1inT_sb, bias1_sb, h1_o, c1_o, h1_in,
        )
        nc.gpsimd.collective_compute(
            "AllGather",
            Alu.bypass,
            replica_groups=[list(range(NCORES))],
            ins=[h1_in.opt()],
            outs=[h1_all.opt()],
        )
        h1all_sb = pc.tile([128, KT_H, B], f32, tag="h1all")
        nc.scalar.dma_start(
            h1all_sb[:], h1_all[:].rearrange("(kt p) b -> p kt b", p=128)
        )
        lstm_scope.close()

        # ---- phase 6: logits = feat @ Wout.T + bout ----------------------
        def feat_tile(kt):
            if kt < 8:
                return xctx_sb[:, kt, :, :]
            if kt < 12:
                return embT_sb[:, kt - 8, :]
            return h1all_sb[:, kt - 12, :]

        with (
            tc.tile_pool(name="out", bufs=1) as pout,
            tc.tile_pool(name="wout", bufs=3) as pwout,
            tc.tile_pool(name="psout", bufs=1, space="PSUM") as pp_out,
        ):
            bout_sb = pout.tile([1, VL], f32, tag="bout")
            nc.sync.dma_start(bout_sb[:], boutv[:].unsqueeze(0))
            psl = pp_out.tile([B, NVC, 512], f32, tag="psl")
            wout_r = woutT[:].rearrange("(kt p) v -> p kt v", p=128)
            for kt in range(KT_F):
                slab = pwout.tile([128, VL], f32, tag="wout", name=f"slab{kt}")
                nc.sync.dma_start(slab[:], wout_r[:, kt, :])
                for vc in range(NVC):
                    n = min(512, VL - vc * 512)
                    nc.tensor.matmul(
                        psl[:, vc, :n],
                        feat_tile(kt),
                        slab[:, vc * 512 : vc * 512 + n],
                        start=(kt == 0),
                        stop=False,
                    )
            log_sb = pout.tile([B, VL], f32, tag="log")
            for vc in range(NVC):
                n = min(512, VL - vc * 512)
                nc.tensor.matmul(
                    psl[:, vc, :n],
                    ones_sb[0:1, 0:B],
                    bout_sb[0:1, vc * 512 : vc * 512 + n],
                    start=False,
                    stop=True,
                )
                nc.scalar.activation(
                    log_sb[:, vc * 512 : vc * 512 + n], psl[:, vc, :n], Act.Copy
                )
            nc.scalar.dma_start(logits_o[:], log_sb[:])

    _split_multi_waits(nc, mybir)
    return nc


_PROGRAM = None


def _get_program():
    global _PROGRAM
    if _PROGRAM is None:
        _PROGRAM = _build_program()
    return _PROGRAM


def _shard_inputs(input_token, hidden, cell, encoder_outputs, mask,
                  embedding, Wq, Wk, v,
                  Wih0, Whh0, bih0, bhh0, Wih1, Whh1, bih1, bhh1,
                  Wout, bout):
    f = np.float32
    asnp = lambda x: np.asarray(x)
    input_token = asnp(input_token)
    hidden = asnp(hidden).astype(f)
    cell = asnp(cell).astype(f)
    encoder_outputs = asnp(encoder_outputs).astype(f)
    mask = asnp(mask)
    embedding = asnp(embedding).astype(f)

    embT = np.ascontiguousarray(embedding[input_token].T)          # [E, B]
    h0inT = np.ascontiguousarray(hidden[0].T)                      # [H, B]
    h1inT = np.ascontiguousarray(hidden[1].T)
    c0T = np.ascontiguousarray(cell[0].T)                          # [H, B]
    c1T = np.ascontiguousarray(cell[1].T)
    wqT = np.ascontiguousarray(asnp(Wq).astype(f).T)               # [H, H]
    wkT = np.ascontiguousarray(asnp(Wk).astype(f).T)               # [ENC, H]
    vv = asnp(v).astype(f)
    Wih0 = asnp(Wih0).astype(f)
    Whh0 = asnp(Whh0).astype(f)
    Wih1 = asnp(Wih1).astype(f)
    Whh1 = asnp(Whh1).astype(f)
    b0 = (asnp(bih0).astype(f) + asnp(bhh0).astype(f))             # [4H]
    b1 = (asnp(bih1).astype(f) + asnp(bhh1).astype(f))
    Wout = asnp(Wout).astype(f)
    bout = asnp(bout).astype(f)

    in_maps = []
    for c in range(NCORES):
        bsl = slice(c * BL, (c + 1) * BL)
        hsl = slice(c * HL, (c + 1) * HL)
        vsl = slice(c * VL, (c + 1) * VL)
        # gate rows owned by this core: 4 slices of HL across i,f,g,o blocks
        grows = np.concatenate(
            [np.arange(gs * H + c * HL, gs * H + (c + 1) * HL) for gs in range(4)]
        )
        wout_c = Wout[vsl]  # [VL, 2560], feat order (h1, ctx, emb)
        woutT_c = np.ascontiguousarray(
            np.concatenate(
                [wout_c[:, H : H + ENC], wout_c[:, H + ENC :], wout_c[:, :H]],
                axis=1,
            ).T
        )  # rows reordered to (ctx, emb, h1)
        in_maps.append({
            "encT": np.ascontiguousarray(
                encoder_outputs[bsl].transpose(2, 0, 1)
            ),
            "maskbias": np.where(
                mask[bsl] == 0, f(NEG), f(0.0)
            ).astype(f).reshape(-1),
            "embT": embT,
            "h0inT": h0inT,
            "h1inT": h1inT,
            "hq1T": np.ascontiguousarray(h1inT[:, c * BL:(c + 1) * BL]),
            "c0inT": np.ascontiguousarray(c0T[hsl]),
            "c1inT": np.ascontiguousarray(c1T[hsl]),
            "wqT": wqT,
            "wkT": wkT,
            "vvec": vv,
            "wih0T": np.ascontiguousarray(Wih0[grows].T),
            "whh0T": np.ascontiguousarray(Whh0[grows].T),
            "wih1T": np.ascontiguousarray(Wih1[grows].T),
            "whh1T": np.ascontiguousarray(Whh1[grows].T),
            "bias0": np.ascontiguousarray(b0[grows].reshape(4, HL).T),
            "bias1": np.ascontiguousarray(b1[grows].reshape(4, HL).T),
            "woutT": woutT_c,
            "boutv": np.ascontiguousarray(bout[vsl]),
        })
    return in_maps


def _run(in_maps, trace=False, **kw):
    from concourse.bass_utils import run_bass_kernel_spmd

    nc = _get_program()
    return run_bass_kernel_spmd(
        nc, in_maps, list(range(NCORES)), trace=trace, **kw
    )


def _assemble(results):
    prediction = np.concatenate(
        [results[c]["logits"] for c in range(NCORES)], axis=1
    )
    attn = np.concatenate([results[c]["attnw"] for c in range(NCORES)], axis=0)
    h0 = np.concatenate([results[c]["h0o"] for c in range(NCORES)], axis=0).T
    h1 = np.concatenate([results[c]["h1o"] for c in range(NCORES)], axis=0).T
    c0 = np.concatenate([results[c]["c0o"] for c in range(NCORES)], axis=0).T
    c1 = np.concatenate([results[c]["c1o"] for c in range(NCORES)], axis=0).T
    new_hidden = np.stack([h0, h1], axis=0)
    new_cell = np.stack([c0, c1], axis=0)
    return prediction, new_hidden, new_cell, attn


def kernel(input_token, hidden, cell, encoder_outputs, mask,
           embedding, Wq, Wk, v,
           Wih0, Whh0, bih0, bhh0, Wih1, Whh1, bih1, bhh1,
           Wout, bout):
    in_maps = _shard_inputs(
        input_token, hidden, cell, encoder_outputs, mask,
        embedding, Wq, Wk, v,
        Wih0, Whh0, bih0, bhh0, Wih1, Whh1, bih1, bhh1,
        Wout, bout,
    )
    res = _run(in_maps)
    return _assemble(res.results)


# revision 27
# speedup vs baseline: 1.6925x; 1.6925x over previous
"""Trainium2 Bass kernel for a seq2seq decoder step (Bahdanau attention +
2-layer LSTM cell + vocab projection), SPMD across 8 NeuronCores.

Sharding (per core c):
  - attention: batch-parallel (8 of 64 batch rows per core)
  - LSTM gates: H-parallel (128 of 1024 hidden units per core, all batches)
  - output projection: vocab-parallel (4000 of 32000 rows per core)
  - AllGather collectives stitch context (batch axis) and h0/h1 (H axis).

All activations are kept feature-major ("T layout", [feature, batch]) so
every matmul contraction lands on the partition axis; all weights are
pre-transposed on the host while sharding (contraction dim first).
"""

import sys
from contextlib import ExitStack

if "/opt/trn_rl_repo" not in sys.path:
    sys.path.insert(0, "/opt/trn_rl_repo")

import numpy as np

V, E, H, ENC, L, B, S = 32000, 512, 1024, 1024, 2, 64, 256
NCORES = 8
BL = B // NCORES        # 8  local batches (attention shard)
HL = H // NCORES        # 128 local hidden slice (LSTM shard)
VL = V // NCORES        # 4000 local vocab slice (output shard)
NEG = -1.0e10


# ---------------------------------------------------------------------------
# Tail-drain workaround: this walrus build rejects >1 sync-wait command on
# CTRL-less instructions (Drain/NoOp). Spread the kernel-tail waits over
# dedicated sync-engine NOPs, one wait each.
# ---------------------------------------------------------------------------
def _patched_tile_context(tile_mod, mybir):
    from concourse.vector_clock import ScopedClock

    class PatchedTileContext(tile_mod.TileContext):
        def _drain_and_barrier(self, tick_clock, wait_clock):
            nc = self.nc
            probe = nc.sync.nop(nofuse=True)
            wait_clock.add_sem_waits(
                probe.ins, ScopedClock({None: tick_clock.global_clock})
            )
            si = probe.ins.sync_info
            waits = list(si.on_wait) if si is not None else []
            updates = list(si.on_update) if si is not None else []
            probe.ins.sync_info = mybir.SyncInfo(
                on_wait=waits[:1], on_update=updates
            )
            for i in range(1, len(waits)):
                n = nc.sync.nop(nofuse=True)
                n.ins.sync_info = mybir.SyncInfo(
                    on_wait=waits[i : i + 1], on_update=[]
                )
            nc.sync.drain()
            nc.all_engine_barrier()
            assert self.sems is not None
            popped = nc._tile_sem_poison_stack.pop()
            assert popped is self._sem_poison
            nc.clear_and_free_semaphores(list(self.sems.allocated().values()))
            nc.all_engine_barrier()

    return PatchedTileContext


# ---------------------------------------------------------------------------
# Program construction (identical on every core; all per-core variation is
# carried by the input data).
# ---------------------------------------------------------------------------
def _split_multi_waits(nc, mybir):
    """This walrus build accepts at most one sync-wait command per
    instruction; Tile attaches several.  Move extra waits onto dedicated
    same-engine NOPs placed immediately before the instruction."""
    for bb in nc.main_func.blocks:
        out = []
        changed = False
        for ins in bb.instructions:
            si = ins.sync_info
            waits = list(si.on_wait) if si is not None else []
            if len(waits) > 1:
                changed = True
                for j, w in enumerate(waits[:-1]):
                    nop = mybir.InstNoOp(
                        name=f"{ins.name}_w{j}",
                        engine=ins.engine,
                        sync_info=mybir.SyncInfo(on_wait=[w], on_update=[]),
                        bass_nofuse=True,
                    )
                    nc.register_instruction(nop, overwrite=True)
                    out.append(nop)
                ins.sync_info = mybir.SyncInfo(
                    on_wait=[waits[-1]], on_update=list(si.on_update)
                )
            out.append(ins)
        if changed:
            bb.instructions = out


def _build_program():
    import concourse.bass as bass
    import concourse.mybir as mybir
    import concourse.tile as tile

    f32 = mybir.dt.float32
    f32r = mybir.dt.float32r
    Alu = mybir.AluOpType
    Act = mybir.ActivationFunctionType
    TC = _patched_tile_context(tile, mybir)

    nc = bass.Bass()

    def mmr(out, lhsT, rhs, **kw):
        # float32r: full-rate single-pass fp32 matmul (plain float32 lowers
        # to two half-speed hi/lo passes with doubled LDWEIGHTS)
        nc.tensor.matmul(out, lhsT, rhs, **kw)

    # ---- per-core inputs -------------------------------------------------
    encT = nc.dram_tensor("encT", [ENC, BL, S], f32, kind="ExternalInput")
    maskbias = nc.dram_tensor("maskbias", [BL * S], f32, kind="ExternalInput")
    embT = nc.dram_tensor("embT", [E, B], f32, kind="ExternalInput")
    h0inT = nc.dram_tensor("h0inT", [H, B], f32, kind="ExternalInput")
    h1inT = nc.dram_tensor("h1inT", [H, B], f32, kind="ExternalInput")
    hq1T = nc.dram_tensor("hq1T", [H, BL], f32, kind="ExternalInput")
    c0inT = nc.dram_tensor("c0inT", [HL, B], f32, kind="ExternalInput")
    c1inT = nc.dram_tensor("c1inT", [HL, B], f32, kind="ExternalInput")
    wqT = nc.dram_tensor("wqT", [H, H], f32, kind="ExternalInput")
    wkT = nc.dram_tensor("wkT", [ENC, H], f32, kind="ExternalInput")
    vvec = nc.dram_tensor("vvec", [H], f32, kind="ExternalInput")
    wih0T = nc.dram_tensor("wih0T", [E + ENC, 4 * HL], f32, kind="ExternalInput")
    whh0T = nc.dram_tensor("whh0T", [H, 4 * HL], f32, kind="ExternalInput")
    wih1T = nc.dram_tensor("wih1T", [H, 4 * HL], f32, kind="ExternalInput")
    whh1T = nc.dram_tensor("whh1T", [H, 4 * HL], f32, kind="ExternalInput")
    bias0 = nc.dram_tensor("bias0", [HL, 4], f32, kind="ExternalInput")
    bias1 = nc.dram_tensor("bias1", [HL, 4], f32, kind="ExternalInput")
    woutT = nc.dram_tensor("woutT", [ENC + E + H, VL], f32, kind="ExternalInput")
    boutv = nc.dram_tensor("boutv", [VL], f32, kind="ExternalInput")

    # ---- per-core outputs ------------------------------------------------
    logits_o = nc.dram_tensor("logits", [B, VL], f32, kind="ExternalOutput")
    attnw_o = nc.dram_tensor("attnw", [BL, S], f32, kind="ExternalOutput")
    h0_o = nc.dram_tensor("h0o", [HL, B], f32, kind="ExternalOutput")
    h1_o = nc.dram_tensor("h1o", [HL, B], f32, kind="ExternalOutput")
    c0_o = nc.dram_tensor("c0o", [HL, B], f32, kind="ExternalOutput")
    c1_o = nc.dram_tensor("c1o", [HL, B], f32, kind="ExternalOutput")

    KT_Q = H // 128        # 8  contraction tiles for Wq/Wk (e dim)
    MT_Q = H // 128        # 8  output tiles for query/keys h dim
    ET = ENC // 128        # 8
    KT_X0 = (E + ENC) // 128   # 12
    KT_H = H // 128        # 8
    KT_F = (ENC + E + H) // 128  # 20 feat tiles, order (ctx, emb, h1)
    NVC = 8                # vocab column chunks of <=512
    GRP = 2                # attention batch groups
    GB = BL // GRP         # 4 batches per group

    with TC(nc) as tc, ExitStack() as root:
        pc = root.enter_context(tc.tile_pool(name="const", bufs=1))
        pdram = root.enter_context(tc.tile_pool(name="dram", bufs=1, space="DRAM"))
        # scopes close in LIFO order: attn first (after phase 3), then lstm
        lstm_scope = ExitStack()
        plstm = lstm_scope.enter_context(tc.tile_pool(name="lstm", bufs=2))
        pwl = lstm_scope.enter_context(tc.tile_pool(name="wl", bufs=4))
        # one PSUM pool for attention + LSTM; every accumulation stream gets
        # its own bank (start=True clears has_written bank-wide on this HW)
        pp_main = lstm_scope.enter_context(
            tc.tile_pool(name="psmain", bufs=2, space="PSUM")
        )
        attn_scope = ExitStack()
        pattn = attn_scope.enter_context(tc.tile_pool(name="attn", bufs=2))
        penc = attn_scope.enter_context(tc.tile_pool(name="enc", bufs=3))

        # ---- phase 0: constants / small loads (sync ring first) ----------
        ones_sb = pc.tile([1, 128], f32, tag="ones")
        nc.vector.memset(ones_sb[:], 1.0)

        mb_sb = pc.tile([1, BL * S], f32, tag="mb")
        nc.sync.dma_start(mb_sb[:], maskbias[:].unsqueeze(0))
        v_sb = pc.tile([128, KT_Q], f32, tag="v")
        nc.sync.dma_start(v_sb[:], vvec[:].rearrange("(kt p) -> p kt", p=128))
        embT_sb = pc.tile([128, E // 128, B], f32r, tag="embT")
        nc.sync.dma_start(
            embT_sb[:], embT[:].rearrange("(kt p) b -> p kt b", p=128).bitcast(f32r)
        )
        h0inT_sb = pc.tile([128, KT_H, B], f32r, tag="h0inT")
        nc.sync.dma_start(
            h0inT_sb[:], h0inT[:].rearrange("(kt p) b -> p kt b", p=128).bitcast(f32r)
        )
        h1inT_sb = pc.tile([128, KT_H, B], f32r, tag="h1inT")
        nc.sync.dma_start(
            h1inT_sb[:], h1inT[:].rearrange("(kt p) b -> p kt b", p=128).bitcast(f32r)
        )
        hq1T_sb = pc.tile([128, KT_H, BL], f32r, tag="hq1T")
        nc.sync.dma_start(
            hq1T_sb[:], hq1T[:].rearrange("(kt p) b -> p kt b", p=128).bitcast(f32r)
        )
        c0inT_sb = pc.tile([HL, B], f32, tag="c0inT")
        nc.sync.dma_start(c0inT_sb[:], c0inT[:])
        c1inT_sb = pc.tile([HL, B], f32, tag="c1inT")
        nc.sync.dma_start(c1inT_sb[:], c1inT[:])
        bias0_sb = pc.tile([HL, 4], f32, tag="bias0")
        nc.sync.dma_start(bias0_sb[:], bias0[:])
        bias1_sb = pc.tile([HL, 4], f32, tag="bias1")
        nc.sync.dma_start(bias1_sb[:], bias1[:])

        # ---- big streaming loads (sync ring, in stream order) ------------
        wk_sb = pc.tile([128, KT_Q, H], f32r, tag="wk")
        nc.sync.dma_start(wk_sb[:], wkT[:].rearrange("(kt p) m -> p kt m", p=128).bitcast(f32r))
        enc_r = encT[:].rearrange("(et p) b s -> p et b s", p=128)
        enc_halves = []
        for gh in range(BL // 2):  # 4 half-tiles of 2 batches
            eh = penc.tile([128, ET, 2, S], f32r, tag="encg", name=f"ench{gh}")
            nc.sync.dma_start(eh[:], enc_r[:, :, 2 * gh : 2 * gh + 2, :].bitcast(f32r))
            enc_halves.append(eh)

        # ---- phase 1: queryT = Wq @ h1inT  -> [H, B] ---------------------
        wq_sb = pattn.tile([128, KT_Q, H], f32r, tag="wq", bufs=1)
        nc.sync.dma_start(wq_sb[:], wqT[:].rearrange("(kt p) m -> p kt m", p=128).bitcast(f32r))
        qT_sb = pc.tile([128, MT_Q, BL], f32, tag="qT")
        for mt in range(MT_Q):
            pq = pp_main.tile([128, BL], f32, tag="pk", bufs=5, name=f"pq{mt}")
            for kt in range(KT_Q):
                mmr(
                    pq[:],
                    wq_sb[:, kt, mt * 128 : (mt + 1) * 128],
                    hq1T_sb[:, kt, :],
                    start=(kt == 0),
                    stop=(kt == KT_Q - 1),
                )
            nc.scalar.activation(qT_sb[:, mt, :], pq[:], Act.Copy)

        # ---- phase 2: attention, 2 groups of 4 batches -------------------
        ctxT_sb = pc.tile([128, ET, BL], f32, tag="ctxT")
        for g in range(GRP):
            pe = [
                pp_main.tile([1, 512], f32, tag="pe", bufs=2, name=f"pe{g}_{i}")
                for i in range(2)
            ]
            # mask bias seeds the energy accumulation
            for p2 in range(2):
                off = (g * GB + 2 * p2) * S
                mmr(
                    pe[p2][:],
                    ones_sb[0:1, 0:1],
                    mb_sb[0:1, off : off + 2 * S],
                    start=True,
                    stop=False,
                )
            for ht in range(MT_Q):
                pk4 = [
                    pp_main.tile([128, S], f32, tag="pk", bufs=5,
                                 name=f"pk{g}_{ht}_{b4}")
                    for b4 in range(GB)
                ]
                for et in range(ET):
                    for b4 in range(GB):
                        eh = enc_halves[g * 2 + b4 // 2]
                        mmr(
                            pk4[b4][:],
                            wk_sb[:, et, ht * 128 : (ht + 1) * 128],
                            eh[:, et, b4 % 2, :],
                            start=(et == 0),
                            stop=(et == ET - 1),
                        )
                tanh_t = pattn.tile([128, GB, S], f32, tag="tanh",
                                    name=f"tanh{g}_{ht}")
                for b4 in range(GB):
                    bloc = g * GB + b4
                    nc.scalar.activation(
                        tanh_t[:, b4, :],
                        pk4[b4][:],
                        Act.Tanh,
                        bias=qT_sb[:, ht, bloc : bloc + 1],
                    )
                for p2 in range(2):
                    mmr(
                        pe[p2][:],
                        v_sb[:, ht : ht + 1],
                        tanh_t[:, 2 * p2 : 2 * p2 + 2, :],
                        start=False,
                        stop=(ht == MT_Q - 1),
                    )
            # softmax over s (masked terms exp to exactly 0)
            mexp = pattn.tile([1, GB, S], f32, tag="mexp", name=f"mexp{g}")
            for b4 in range(GB):
                nc.scalar.activation(
                    mexp[0:1, b4, :],
                    pe[b4 // 2][0:1, (b4 % 2) * S : (b4 % 2 + 1) * S],
                    Act.Exp,
                )
            sums = pattn.tile([1, GB], f32, tag="sums", name=f"sums{g}")
            nc.vector.tensor_reduce(
                sums[:], mexp[:], mybir.AxisListType.X, Alu.add
            )
            recip = pattn.tile([1, GB], f32, tag="recip", name=f"recip{g}")
            nc.vector.reciprocal(recip[:], sums[:])
            atw = pattn.tile([1, GB, S], f32, tag="atw", name=f"atw{g}")
            for b4 in range(GB):
                nc.vector.tensor_scalar_mul(
                    atw[0:1, b4, :],
                    mexp[0:1, b4, :],
                    recip[0:1, b4 : b4 + 1],
                )
            nc.scalar.dma_start(
                attnw_o[g * GB : (g + 1) * GB, :].unsqueeze(0), atw[:]
            )
            # context: ctxT[:, b] = sum_s encT[:, s] * attnw[s]
            for b4 in range(GB):
                bloc = g * GB + b4
                pbc = pp_main.tile([128, S], f32, tag="pbc", bufs=1,
                                   name=f"pbc{bloc}")
                mmr(
                    pbc[:], ones_sb[0:1, :], atw[0:1, b4, :],
                    start=True, stop=True,
                )
                atw_bc = pattn.tile([128, S], f32, tag="atwbc",
                                    name=f"atwbc{bloc}")
                nc.scalar.activation(atw_bc[:], pbc[:], Act.Copy)
                eh = enc_halves[g * 2 + b4 // 2]
                for et in range(ET):
                    scr = pattn.tile([128, S], f32, tag="scr",
                                     name=f"scr{bloc}_{et}")
                    nc.vector.tensor_tensor(
                        scr[:], eh[:, et, b4 % 2, :].bitcast(f32), atw_bc[:],
                        Alu.mult
                    )
                    nc.vector.tensor_reduce(
                        ctxT_sb[:, et, bloc : bloc + 1],
                        scr[:],
                        mybir.AxisListType.X,
                        Alu.add,
                    )

        # ---- phase 3: AllGather context over batch -----------------------
        ctx_in = pdram.tile([ENC, BL], f32, tag="ctx_in")
        ctx_all = pdram.tile([NCORES * ENC, BL], f32, tag="ctx_all")
        nc.scalar.dma_start(
            ctx_in[:].rearrange("(et p) b -> p et b", p=128), ctxT_sb[:]
        )
        nc.gpsimd.collective_compute(
            "AllGather",
            Alu.bypass,
            replica_groups=[list(range(NCORES))],
            ins=[ctx_in.opt()],
            outs=[ctx_all.opt()],
        )
        xctx_sb = pc.tile([128, ET, NCORES, BL], f32r, tag="xctx")
        ctx_all_r = ctx_all[:].rearrange("(r et p) b -> p et r b", p=128, et=ET)
        for et in range(ET):
            nc.scalar.dma_start(xctx_sb[:, et, :, :], ctx_all_r[:, et, :, :].bitcast(f32r))
        # attention scratch no longer needed; free its SBUF/PSUM for phase 6
        attn_scope.close()

        # ---- phase 4/5: the two LSTM layers (H-sharded gates) ------------
        def lstm_layer(lname, wihT_d, kt_ih, x_tiles, whhT_d, hin_sb, cin_sb,
                       bias_sb, h_out_dram, c_out_dram, h_bounce):
            psg = [
                pp_main.tile([128, B], f32, tag="pk", bufs=5,
                             name=f"psg{lname}_{gs}")
                for gs in range(4)
            ]
            wih_r = wihT_d[:].rearrange("(kt p) m -> p kt m", p=128)
            whh_r = whhT_d[:].rearrange("(kt p) m -> p kt m", p=128)
            for kt in range(kt_ih + KT_H):
                wl = pwl.tile([128, 4 * HL], f32r, tag="wl",
                              name=f"wl{lname}_{kt}")
                # Whh-part first: its rhs (previous hidden state) is a kernel
                # input, so these matmuls overlap the collective that feeds
                # the Wih-part's rhs.
                if kt < KT_H:
                    nc.sync.dma_start(wl[:], whh_r[:, kt, :].bitcast(f32r))
                    rhs = hin_sb[:, kt, :]
                else:
                    nc.sync.dma_start(wl[:], wih_r[:, kt - KT_H, :].bitcast(f32r))
                    rhs = x_tiles(kt - KT_H)
                for gs in range(4):
                    mmr(
                        psg[gs][:],
                        wl[:, gs * HL : (gs + 1) * HL],
                        rhs,
                        start=(kt == 0),
                        stop=(kt == kt_ih + KT_H - 1),
                    )
            gi = plstm.tile([128, B], f32, tag="gi", name=f"gi{lname}")
            gf = plstm.tile([128, B], f32, tag="gf", name=f"gf{lname}")
            gg = plstm.tile([128, B], f32, tag="gg", name=f"gg{lname}")
            go = plstm.tile([128, B], f32, tag="go", name=f"go{lname}")
            nc.scalar.activation(gi[:], psg[0][:], Act.Sigmoid,
                                 bias=bias_sb[:, 0:1])
            nc.scalar.activation(gf[:], psg[1][:], Act.Sigmoid,
                                 bias=bias_sb[:, 1:2])
            nc.scalar.activation(gg[:], psg[2][:], Act.Tanh,
                                 bias=bias_sb[:, 2:3])
            nc.scalar.activation(go[:], psg[3][:], Act.Sigmoid,
                                 bias=bias_sb[:, 3:4])
            fc = plstm.tile([128, B], f32, tag="fc", name=f"fc{lname}")
            nc.vector.tensor_tensor(fc[:], gf[:], cin_sb[:], Alu.mult)
            ig = plstm.tile([128, B], f32, tag="ig", name=f"ig{lname}")
            nc.vector.tensor_tensor(ig[:], gi[:], gg[:], Alu.mult)
            cT = plstm.tile([128, B], f32, tag="cT", name=f"cT{lname}")
            nc.vector.tensor_tensor(cT[:], fc[:], ig[:], Alu.add)
            tc_ = plstm.tile([128, B], f32, tag="tc_", name=f"tc{lname}")
            nc.scalar.activation(tc_[:], cT[:], Act.Tanh)
            hT = plstm.tile([128, B], f32, tag="hT", name=f"hT{lname}")
            nc.vector.tensor_tensor(hT[:], go[:], tc_[:], Alu.mult)
            nc.scalar.dma_start(c_out_dram[:], cT[:])
            nc.scalar.dma_start(h_out_dram[:], hT[:])
            nc.scalar.dma_start(h_bounce[:], hT[:])

        h0_in = pdram.tile([HL, B], f32, tag="h0_in")
        h0_all = pdram.tile([H, B], f32, tag="h0_all")
        lstm_layer(
            "0", wih0T, KT_X0,
            lambda kt: embT_sb[:, kt, :] if kt < 4
            else xctx_sb[:, kt - 4, :, :],
            whh0T, h0inT_sb, c0inT_sb, bias0_sb, h0_o, c0_o, h0_in,
        )
        nc.gpsimd.collective_compute(
            "AllGather",
            Alu.bypass,
            replica_groups=[list(range(NCORES))],
            ins=[h0_in.opt()],
            outs=[h0_all.opt()],
        )
        h0all_sb = pc.tile([128, KT_H, B], f32r, tag="h0all")
        nc.scalar.dma_start(
            h0all_sb[:], h0_all[:].rearrange("(kt p) b -> p kt b", p=128).bitcast(f32r)
        )

        h1_in = pdram.tile([HL, B], f32, tag="h1_in")
        h1_all = pdram.tile([H, B], f32, tag="h1_all")
        lstm_layer(
            "1", wih1T, KT_H,
            lambda kt: h0all_sb[:, kt, :],
            whh1T, h1inT_sb, c1inT_sb, bias1_sb, h1_o, c1_o, h1_in,
        )
        nc.gpsimd.collective_compute(
            "AllGather",
            Alu.bypass,
            replica_groups=[list(range(NCORES))],
            ins=[h1_in.opt()],
            outs=[h1_all.opt()],
        )
        h1all_sb = pc.tile([128, KT_H, B], f32r, tag="h1all")
        nc.scalar.dma_start(
            h1all_sb[:], h1_all[:].rearrange("(kt p) b -> p kt b", p=128).bitcast(f32r)
        )
        lstm_scope.close()

        # ---- phase 6: logits = feat @ Wout.T + bout ----------------------
        def feat_tile(kt):
            if kt < 8:
                return xctx_sb[:, kt, :, :]
            if kt < 12:
                return embT_sb[:, kt - 8, :]
            return h1all_sb[:, kt - 12, :]

        with (
            tc.tile_pool(name="out", bufs=1) as pout,
            tc.tile_pool(name="wout", bufs=3) as pwout,
            tc.tile_pool(name="psout", bufs=1, space="PSUM") as pp_out,
        ):
            bout_sb = pout.tile([1, VL], f32, tag="bout")
            nc.scalar.dma_start(bout_sb[:], boutv[:].unsqueeze(0))
            psl = pp_out.tile([B, NVC, 512], f32, tag="psl")
            wout_r = woutT[:].rearrange("(kt p) v -> p kt v", p=128)
            for kt in range(KT_F):
                slab = pwout.tile([128, VL], f32r, tag="wout", name=f"slab{kt}")
                nc.scalar.dma_start(slab[:], wout_r[:, kt, :].bitcast(f32r))
                for vc in range(NVC):
                    n = min(512, VL - vc * 512)
                    mmr(
                        psl[:, vc, :n],
                        feat_tile(kt),
                        slab[:, vc * 512 : vc * 512 + n],
                        start=(kt == 0),
                        stop=False,
                    )
            log_sb = pout.tile([B, VL], f32, tag="log")
            for vc in range(NVC):
                n = min(512, VL - vc * 512)
                mmr(
                    psl[:, vc, :n],
                    ones_sb[0:1, 0:B],
                    bout_sb[0:1, vc * 512 : vc * 512 + n],
                    start=False,
                    stop=True,
                )
                nc.scalar.activation(
                    log_sb[:, vc * 512 : vc * 512 + n], psl[:, vc, :n], Act.Copy
                )
            nc.scalar.dma_start(logits_o[:], log_sb[:])

    _split_multi_waits(nc, mybir)
    return nc


_PROGRAM = None


def _get_program():
    global _PROGRAM
    if _PROGRAM is None:
        _PROGRAM = _build_program()
    return _PROGRAM


def _shard_inputs(input_token, hidden, cell, encoder_outputs, mask,
                  embedding, Wq, Wk, v,
                  Wih0, Whh0, bih0, bhh0, Wih1, Whh1, bih1, bhh1,
                  Wout, bout):
    f = np.float32
    asnp = lambda x: np.asarray(x)
    input_token = asnp(input_token)
    hidden = asnp(hidden).astype(f)
    cell = asnp(cell).astype(f)
    encoder_outputs = asnp(encoder_outputs).astype(f)
    mask = asnp(mask)
    embedding = asnp(embedding).astype(f)

    embT = np.ascontiguousarray(embedding[input_token].T)          # [E, B]
    h0inT = np.ascontiguousarray(hidden[0].T)                      # [H, B]
    h1inT = np.ascontiguousarray(hidden[1].T)
    c0T = np.ascontiguousarray(cell[0].T)                          # [H, B]
    c1T = np.ascontiguousarray(cell[1].T)
    wqT = np.ascontiguousarray(asnp(Wq).astype(f).T)               # [H, H]
    wkT = np.ascontiguousarray(asnp(Wk).astype(f).T)               # [ENC, H]
    vv = asnp(v).astype(f)
    Wih0 = asnp(Wih0).astype(f)
    Whh0 = asnp(Whh0).astype(f)
    Wih1 = asnp(Wih1).astype(f)
    Whh1 = asnp(Whh1).astype(f)
    b0 = (asnp(bih0).astype(f) + asnp(bhh0).astype(f))             # [4H]
    b1 = (asnp(bih1).astype(f) + asnp(bhh1).astype(f))
    Wout = asnp(Wout).astype(f)
    bout = asnp(bout).astype(f)

    in_maps = []
    for c in range(NCORES):
        bsl = slice(c * BL, (c + 1) * BL)
        hsl = slice(c * HL, (c + 1) * HL)
        vsl = slice(c * VL, (c + 1) * VL)
        # gate rows owned by this core: 4 slices of HL across i,f,g,o blocks
        grows = np.concatenate(
            [np.arange(gs * H + c * HL, gs * H + (c + 1) * HL) for gs in range(4)]
        )
        wout_c = Wout[vsl]  # [VL, 2560], feat order (h1, ctx, emb)
        woutT_c = np.ascontiguousarray(
            np.concatenate(
                [wout_c[:, H : H + ENC], wout_c[:, H + ENC :], wout_c[:, :H]],
                axis=1,
            ).T
        )  # rows reordered to (ctx, emb, h1)
        in_maps.append({
            "encT": np.ascontiguousarray(
                encoder_outputs[bsl].transpose(2, 0, 1)
            ),
            "maskbias": np.where(
                mask[bsl] == 0, f(NEG), f(0.0)
            ).astype(f).reshape(-1),
            "embT": embT,
            "h0inT": h0inT,
            "h1inT": h1inT,
            "hq1T": np.ascontiguousarray(h1inT[:, c * BL:(c + 1) * BL]),
            "c0inT": np.ascontiguousarray(c0T[hsl]),
            "c1inT": np.ascontiguousarray(c1T[hsl]),
            "wqT": wqT,
            "wkT": wkT,
            "vvec": vv,
            "wih0T": np.ascontiguousarray(Wih0[grows].T),
            "whh0T": np.ascontiguousarray(Whh0[grows].T),
            "wih1T": np.ascontiguousarray(Wih1[grows].T),
            "whh1T": np.ascontiguousarray(Whh1[grows].T),
            "bias0": np.ascontiguousarray(b0[grows].reshape(4, HL).T),
            "bias1": np.ascontiguousarray(b1[grows].reshape(4, HL).T),
            "woutT": woutT_c,
            "boutv": np.ascontiguousarray(bout[vsl]),
        })
    return in_maps


def _run(in_maps, trace=False, **kw):
    from concourse.bass_utils import run_bass_kernel_spmd

    nc = _get_program()
    return run_bass_kernel_spmd(
        nc, in_maps, list(range(NCORES)), trace=trace, **kw
    )


def _assemble(results):
    prediction = np.concatenate(
        [results[c]["logits"] for c in range(NCORES)], axis=1
    )
    attn = np.concatenate([results[c]["attnw"] for c in range(NCORES)], axis=0)
    h0 = np.concatenate([results[c]["h0o"] for c in range(NCORES)], axis=0).T
    h1 = np.concatenate([results[c]["h1o"] for c in range(NCORES)], axis=0).T
    c0 = np.concatenate([results[c]["c0o"] for c in range(NCORES)], axis=0).T
    c1 = np.concatenate([results[c]["c1o"] for c in range(NCORES)], axis=0).T
    new_hidden = np.stack([h0, h1], axis=0)
    new_cell = np.stack([c0, c1], axis=0)
    return prediction, new_hidden, new_cell, attn


def kernel(input_token, hidden, cell, encoder_outputs, mask,
           embedding, Wq, Wk, v,
           Wih0, Whh0, bih0, bhh0, Wih1, Whh1, bih1, bhh1,
           Wout, bout):
    in_maps = _shard_inputs(
        input_token, hidden, cell, encoder_outputs, mask,
        embedding, Wq, Wk, v,
        Wih0, Whh0, bih0, bhh0, Wih1, Whh1, bih1, bhh1,
        Wout, bout,
    )
    res = _run(in_maps)
    return _assemble(res.results)
